# revision 8
# baseline (speedup 1.0000x reference)
"""Trainium2 Bass kernel for nn_CHSHistoryCrossAttentionFusion (8 NeuronCores, SPMD).

Decomposition (hardcoded for B=2, S=4096, L=3, D=1024, N=512, 8 cores):
  - Batch-split history sharding: cores 0-3 own batch 0, cores 4-7 batch 1;
    core c owns key positions [(c%4)*1024, (c%4+1)*1024) of its batch and
    computes fused/K for that chunk from its x strips.
  - Queries: core c owns the 128 queries (c%4)*128..+128 of its batch; it
    computes their fused/Q projection, then a 4-core-group AllGather (groups
    [0..3] / [4..7] run concurrently) replicates the *pre-transposed* Q so
    every core scores all 512 of its batch's queries against its K chunk.
  - Flash-style partial softmax per chunk WITHOUT max subtraction (Q/K are
    RMS-normalized so scores are bounded); causal mask applied additively
    before exp; exp carries a constant -ln(256) prescale so the (o,l)
    partials fit fp16.  Wv is factored OUT of the partial accumulation:
    partials are attn@fused (not attn@V), combined by one fp16 4-core-group
    ReduceScatter, and Wv/Wo are applied to the core's own 128 queries only.
  - x is supplied host-side as 9 pre-transposed, partition-reblocked
    [128, 24*128] strips (8 history tiles + 1 gathered query tile) so the
    fc matmul needs no on-chip transposes; strips stream f32->bf16 through
    the single SWDGE cast queue (wo follows them).  Dense weights stream
    f32 on the two HWDGE queues (sync+scalar) and are cast to bf16 on
    vector/gpsimd/scalar, ordered so wfc+wq land first (fc + AllGather
    critical path), then wk for the K phase, then wv/wo for the epilogue.
    Cast/collective emission is placed so no in-order engine queue blocks
    on data it doesn't need yet; the Q epilogue is split so the fps_t8
    PSUM drains before group B while the Wq matmul waits for wq casts.
  - All matmuls bf16 (fp32 accumulate); f32 in/out.
Host-side work is layout/indexing only (the sinusoidal position table is
host-precomputed as in the reference construction, shipped bf16).
"""

import math
import os

import numpy as np

try:
    import ml_dtypes
except ImportError:  # pragma: no cover
    ml_dtypes = None

import concourse.bacc as bacc
import concourse.mybir as mybir
import concourse.tile as tile
import concourse.tile_utils as tile_utils
from concourse.bass_utils import run_bass_kernel_spmd

# cayman has 208 KiB/partition usable; the default constant leaves 16 KiB idle
tile_utils.max_sbuf_usage = 208 * 1024

F32 = mybir.dt.float32
F16 = mybir.dt.float16
BF16 = mybir.dt.bfloat16
AF = mybir.ActivationFunctionType
OP = mybir.AluOpType

B, S, L, D = 2, 4096, 3, 1024
N = 512
NC = 8
GRP = 4                   # collective group size (one batch per group)
CH = S // GRP             # 1024 keys per core (one batch)
LD = L * D                # 3072
QPC = 128                 # queries owned per core
NQT = N // QPC            # 4 query tiles per batch
NKK = LD // 128           # 24 contraction slices over 3072
NJ = D // 128             # 8 contraction slices over 1024
NT = 9                    # 8 history tiles + 1 query tile
RMS_EPS = 1e-6
SCALE = D ** -0.5
MASK_NEG = -1.0e6
EXP_BIAS = -math.log(256.0)

_CACHE = {}


def _build(apply_norm_weights: bool):
    nc = bacc.Bacc("TRN2", target_bir_lowering=False, num_devices=NC)

    # ---------------- I/O ----------------
    x3 = nc.dram_tensor("x3", [NT * 128, NKK * 128], F32, kind="ExternalInput")
    wfc = nc.dram_tensor("wfc", [LD, D], F32, kind="ExternalInput")
    wq = nc.dram_tensor("wq", [D, D], F32, kind="ExternalInput")
    wk = nc.dram_tensor("wk", [D, D], F32, kind="ExternalInput")
    wv = nc.dram_tensor("wv", [D, D], F32, kind="ExternalInput")
    wo = nc.dram_tensor("wo", [D, D], F32, kind="ExternalInput")
    pet = nc.dram_tensor("pet", [8 * D, 128], BF16, kind="ExternalInput")
    peq = nc.dram_tensor("peq", [QPC, D], BF16, kind="ExternalInput")
    thr = nc.dram_tensor("thr", [128, NQT], F32, kind="ExternalInput")
    iota = nc.dram_tensor("iota", [128, CH], F16, kind="ExternalInput")
    ident = nc.dram_tensor("ident", [128, 128], BF16, kind="ExternalInput")
    if apply_norm_weights:
        whn = nc.dram_tensor("whn", [128, D], F32, kind="ExternalInput")
        wqn = nc.dram_tensor("wqn", [128, D], F32, kind="ExternalInput")
        wkn = nc.dram_tensor("wkn", [128, D], F32, kind="ExternalInput")
        won = nc.dram_tensor("won", [128, D], F32, kind="ExternalInput")
    out = nc.dram_tensor("out", [QPC, D], F32, kind="ExternalOutput")

    GROUPS = [[0, 1, 2, 3], [4, 5, 6, 7]]

    with tile.TileContext(nc) as tc:
        with (
            tc.tile_pool(name="dram", bufs=1, space="DRAM") as dram,
            tc.tile_pool(name="const", bufs=1) as constp,
            tc.tile_pool(name="stat", bufs=6) as stat,
            tc.tile_pool(name="wpool", bufs=1) as wpool,
            tc.tile_pool(name="stage", bufs=2) as stagep,
            tc.tile_pool(name="strip", bufs=5) as stripp,
            tc.tile_pool(name="base", bufs=1) as base,
            tc.tile_pool(name="scr_bf", bufs=2) as scr_bf,
            tc.tile_pool(name="scr_f", bufs=2) as scr_f,
            tc.tile_pool(name="mmps", bufs=3, space="PSUM") as mmps,
            tc.tile_pool(name="trps", bufs=2, space="PSUM") as trps,
        ):
            # collective bounce buffers
            ag_in = dram.tile([D, QPC], BF16)
            ag_out = dram.tile([GRP * D, QPC], BF16)
            rs_in = dram.tile([N, D + 1], F16)
            rs_out = dram.tile([QPC, D + 1], F16)

            # ---- small constants (sync queue, land first) ----
            id_sb = constp.tile([128, 128], BF16)
            nc.sync.dma_start(id_sb[:], ident[:])
            iota_sb = constp.tile([128, CH], F16)
            nc.sync.dma_start(iota_sb[:], iota[:])
            thr_sb = constp.tile([128, NQT], F32)
            nc.sync.dma_start(thr_sb[:], thr[:])
            eps_sb = constp.tile([128, 1], F32)
            nc.vector.memset(eps_sb[:], RMS_EPS)
            ebias_sb = constp.tile([128, 1], F32)
            nc.vector.memset(ebias_sb[:], EXP_BIAS)
            if apply_norm_weights:
                whn_sb = constp.tile([128, D], F32)
                nc.sync.dma_start(whn_sb[:], whn[:])
                wqn_sb = constp.tile([128, D], F32)
                nc.sync.dma_start(wqn_sb[:], wqn[:])
                wkn_sb = constp.tile([128, D], F32)
                nc.sync.dma_start(wkn_sb[:], wkn[:])
                won_sb = constp.tile([128, D], F32)
                nc.sync.dma_start(won_sb[:], won[:])

            # peq (bf16, tiny) first on the scalar HWDGE queue
            peq_bf = wpool.tile([QPC, D], BF16)
            nc.scalar.dma_start(peq_bf[:], peq.ap())

            # ---- bulk loads ----
            # SWDGE (gpsimd cast queue): x strips, query strip first.
            strips = [None] * 8

            def load_strip(t, name):
                st = stripp.tile([128, NKK * 128], BF16, tag="strip",
                                 name=name)
                idx = 0 if t is None else 1 + t
                nc.gpsimd.dma_start(
                    st[:], x3.ap()[idx * 128:(idx + 1) * 128, :])
                return st

            strip_q = load_strip(None, "strip_q")
            for t in range(8):
                strips[t] = load_strip(t, f"strip{t}")

            # HWDGE f32 loads + engine casts.
            wfc_bf = wpool.tile([128, NKK * D], BF16)

            def wfc_slice(kk):
                stg = stagep.tile([128, D], F32, tag="stg", name=f"wfst{kk}")
                eng = nc.sync if kk % 2 == 0 else nc.scalar
                eng.dma_start(stg[:], wfc.ap()[kk * 128:(kk + 1) * 128, :])
                nc.vector.tensor_copy(wfc_bf[:, kk * D:(kk + 1) * D], stg[:])

            for kk in range(NKK):
                wfc_slice(kk)

            def hw_w_dma(src, nm, s_, eng):
                stg = stagep.tile([128, D], F32, tag="stg", name=f"{nm}st{s_}")
                eng.dma_start(stg[:], src.ap()[s_ * 128:(s_ + 1) * 128, :])
                return stg

            # wq: split across both HWDGE queues; cast on gpsimd.
            wq_sb = wpool.tile([128, NJ * D], BF16, tag="wqv", name="wq_sb")
            for s_ in range(NJ):
                stg = hw_w_dma(wq, "wq", s_,
                               nc.sync if s_ % 2 == 0 else nc.scalar)
                nc.gpsimd.tensor_copy(wq_sb[:, s_ * D:(s_ + 1) * D], stg[:])

            # persistent activations
            fusedT = base.tile([128, NJ * CH], BF16, name="fusedT")
            fusedT_v = fusedT[:].rearrange("p (j t) -> p j t", j=NJ)
            fnat = base.tile([128, 8 * D], BF16, name="fnat")
            qs_bf = base.tile([QPC, D], BF16)
            kT = base.tile([128, NJ * CH], BF16, name="kT")
            kT_v = kT[:].rearrange("p (j t) -> p j t", j=NJ)

            def rms_stats(src_ap):
                sq = scr_f.tile([128, D], F32, tag="sqscr")
                ssq = stat.tile([128, 1], F32, tag="ssq")
                nc.scalar.activation(sq[:], src_ap, AF.Square, accum_out=ssq[:])
                std = stat.tile([128, 1], F32, tag="std")
                nc.scalar.activation(std[:], ssq[:], AF.Sqrt, scale=1.0 / D,
                                     bias=eps_sb[:])
                rstd = stat.tile([128, 1], F32, tag="rstd")
                nc.vector.reciprocal(rstd[:], std[:])
                return rstd

            def transpose_to(dst_ap_3d, src_tile_ap, jlist):
                """PE-transpose 128x128 blocks into dst 3d view [128,len,128]."""
                ps = trps.tile([128, 512], BF16, tag="trp")
                for u, j in enumerate(jlist):
                    nc.tensor.transpose(
                        ps[:, u * 128:(u + 1) * 128],
                        src_tile_ap[:, j * 128:(j + 1) * 128],
                        id_sb[:],
                    )
                nc.vector.tensor_copy(
                    dst_ap_3d,
                    ps[:].rearrange("p (u x) -> p u x", u=len(jlist)),
                )

            def transpose_full(dst_tile, src_ap):
                """dst[:, j*128:+128] = src[:, j*128:+128].T for all 8 j."""
                dst_v = dst_tile[:].rearrange("p (g x) -> p g x", g=2)
                for g in range(2):
                    transpose_to(
                        dst_v[:, g:g + 1, :].rearrange("p g x -> p (g x)")
                        .rearrange("p (u x) -> p u x", u=4),
                        src_ap,
                        [g * 4 + u for u in range(4)],
                    )

            # ---------------- phase 1: fc matmul ----------
            fps_tiles = {}

            def fc_group(tiles):
                """kk-outer over a group (used while wfc streams in)."""
                for key, _ in tiles:
                    fps_tiles[key] = mmps.tile([128, D], F32, tag="mm",
                                               name=f"fps{key}")
                for kk in range(NKK):
                    for key, st in tiles:
                        fps = fps_tiles[key]
                        for h in range(2):
                            nc.tensor.matmul(
                                fps[:, h * 512:(h + 1) * 512],
                                st[:, kk * 128:(kk + 1) * 128],
                                wfc_bf[:, kk * D + h * 512: kk * D + h * 512 + 512],
                                start=(kk == 0),
                                stop=(kk == NKK - 1),
                            )

            def fused_epilogue(t):
                fps = fps_tiles[t]
                rstd = rms_stats(fps[:])
                fb = fnat[:, t * D:(t + 1) * D]
                nc.vector.tensor_scalar(fb, fps[:], rstd[:], None, OP.mult)
                if apply_norm_weights:
                    nc.vector.tensor_tensor(fb, fb, whn_sb[:], op=OP.mult)
                for g in range(2):
                    transpose_to(
                        fusedT_v[:, g * 4:(g + 1) * 4,
                                 t * 128:(t + 1) * 128],
                        fb,
                        [g * 4 + u for u in range(4)],
                    )

            qep = {}

            def q_epilogue_a():
                """Drains fps_t8: rms + positioned qT; no wq dependency."""
                fps = fps_tiles["t8"]
                rstd = rms_stats(fps[:])
                nc.vector.tensor_scalar(qs_bf[:], fps[:], rstd[:], None,
                                        OP.mult)
                if apply_norm_weights:
                    nc.vector.tensor_tensor(qs_bf[:], qs_bf[:], whn_sb[:],
                                            op=OP.mult)
                qhb = scr_bf.tile([128, D], BF16, tag="tmb")
                nc.vector.tensor_scalar(qhb[:], fps[:], rstd[:], None, OP.mult)
                if apply_norm_weights:
                    nc.vector.tensor_tensor(qhb[:], qhb[:], whn_sb[:],
                                            op=OP.mult)
                nc.vector.tensor_add(qhb[:], qhb[:], peq_bf[:])
                qht = scr_bf.tile([128, D], BF16, tag="tmb")
                transpose_full(qht, qhb[:])
                qep["qht"] = qht

            def q_epilogue_b():
                """Wq matmul + rms + own-qT AllGather (needs wq casts)."""
                qht = qep["qht"]
                qps = mmps.tile([128, D], F32, tag="mm", name="qps")
                for j in range(NJ):
                    for h in range(2):
                        nc.tensor.matmul(
                            qps[:, h * 512:(h + 1) * 512],
                            qht[:, j * 128:(j + 1) * 128],
                            wq_sb[:, j * D + h * 512: j * D + h * 512 + 512],
                            start=(j == 0),
                            stop=(j == NJ - 1),
                        )
                qrstd = rms_stats(qps[:])
                qb = scr_bf.tile([128, D], BF16, tag="tmb")
                nc.vector.tensor_scalar(qb[:], qps[:], qrstd[:], None, OP.mult)
                if apply_norm_weights:
                    nc.vector.tensor_tensor(qb[:], qb[:], wqn_sb[:],
                                            op=OP.mult)
                qt = scr_bf.tile([128, D], BF16, tag="tmb")
                transpose_full(qt, qb[:])
                nc.sync.dma_start(
                    ag_in.rearrange("(j p) q -> p j q", p=128),
                    qt[:].rearrange("p (j q) -> p j q", j=NJ))
                nc.gpsimd.collective_compute(
                    "AllGather", OP.bypass,
                    replica_groups=GROUPS,
                    ins=[ag_in.opt()],
                    outs=[ag_out.opt()],
                )

            # -------- phase 2 helpers --------
            def k_tile(tl, wk_sb):
                pts = scr_bf.tile([128, NJ * 128], BF16, tag="pts", bufs=2)
                nc.sync.dma_start(
                    pts[:].rearrange("p (j t) -> p j t", j=NJ),
                    pet.ap()[tl * D:(tl + 1) * D, :]
                    .rearrange("(j p) t -> p j t", p=128))
                khb = scr_bf.tile([128, NJ * 128], BF16, tag="khb", bufs=1)
                nc.vector.tensor_add(
                    khb[:].rearrange("p (j x) -> p j x", j=NJ),
                    fusedT_v[:, :, tl * 128:(tl + 1) * 128],
                    pts[:].rearrange("p (j t) -> p j t", j=NJ),
                )
                kps = mmps.tile([128, D], F32, tag="mm")
                for j in range(NJ):
                    for h in range(2):
                        nc.tensor.matmul(
                            kps[:, h * 512:(h + 1) * 512],
                            khb[:, j * 128:(j + 1) * 128],
                            wk_sb[:, j * D + h * 512: j * D + h * 512 + 512],
                            start=(j == 0),
                            stop=(j == NJ - 1),
                        )
                krstd = rms_stats(kps[:])
                kb = scr_bf.tile([128, D], BF16, tag="tmb")
                nc.vector.tensor_scalar(kb[:], kps[:], krstd[:], None, OP.mult)
                if apply_norm_weights:
                    nc.vector.tensor_tensor(kb[:], kb[:], wkn_sb[:],
                                            op=OP.mult)
                for g in range(2):
                    transpose_to(
                        kT_v[:, g * 4:(g + 1) * 4, tl * 128:(tl + 1) * 128],
                        kb[:],
                        [g * 4 + u for u in range(4)],
                    )

            def attn_tile(i):
                # load pre-transposed q tile i straight from the AllGather
                qTt = scr_bf.tile([128, NJ * 128], BF16, tag="qTt", bufs=1,
                                  name=f"qT{i}")
                nc.sync.dma_start(
                    qTt[:].rearrange("p (j q) -> p j q", j=NJ),
                    ag_out[i * D:(i + 1) * D, :]
                    .rearrange("(j p) q -> p j q", p=128))
                sps = mmps.tile([128, CH], F32, tag="mm")
                for j in range(NJ):
                    for h in range(2):
                        nc.tensor.matmul(
                            sps[:, h * 512:(h + 1) * 512],
                            qTt[:, j * 128:(j + 1) * 128],
                            kT[:, j * CH + h * 512: j * CH + h * 512 + 512],
                            start=(j == 0),
                            stop=(j == NJ - 1),
                        )
                mb = scr_f.tile([128, CH], F32, tag="mb", bufs=1)
                nc.vector.tensor_scalar(mb[:], iota_sb[:],
                                        thr_sb[:, i:i + 1], MASK_NEG,
                                        OP.is_gt, OP.mult)
                nc.vector.tensor_add(mb[:], mb[:], sps[:])
                o_sb = scr_f.tile([128, D + 1], F16, tag="osb", bufs=2)
                lacc = stat.tile([128, 1], F32, tag="lacc")
                probs = scr_bf.tile([128, CH], BF16, tag="probs", bufs=1)
                nc.scalar.activation(probs[:], mb[:], AF.Exp, scale=SCALE,
                                     bias=ebias_sb[:], accum_out=lacc[:])
                nc.scalar.copy(o_sb[:, D:D + 1], lacc[:])
                pT = scr_bf.tile([128, NJ * 128], BF16, tag="pT", bufs=1)
                pT_v = pT[:].rearrange("p (u x) -> p u x", u=NJ)
                for g in range(2):
                    transpose_to(
                        pT_v[:, g * 4:(g + 1) * 4, :],
                        probs[:, g * 512:(g + 1) * 512],
                        list(range(4)),
                    )
                ops_ = mmps.tile([128, D], F32, tag="mm")
                for u in range(8):
                    for h in range(2):
                        nc.tensor.matmul(
                            ops_[:, h * 512:(h + 1) * 512],
                            pT[:, u * 128:(u + 1) * 128],
                            fnat[:, u * D + h * 512: u * D + h * 512 + 512],
                            start=(u == 0),
                            stop=(u == 7),
                        )
                nc.vector.tensor_copy(o_sb[:, 0:D], ops_[:])
                nc.scalar.dma_start(rs_in[i * 128:(i + 1) * 128, :], o_sb[:])

            # ---------------- emission schedule ----------------
            # fc groups 3+3+2+1 (PSUM: 3 tiles max + trps + qps slot reuse).
            fc_group([("t8", strip_q), (0, strips[0]), (1, strips[1])])
            q_epilogue_a()        # drains fps_t8 (no wq dependency)
            fused_epilogue(0)
            fused_epilogue(1)
            fc_group([(2, strips[2]), (3, strips[3]), (4, strips[4])])
            fused_epilogue(2)
            q_epilogue_b()        # Wq matmul + AllGather trigger (gpsimd)
            fused_epilogue(3)
            fused_epilogue(4)

            # wk: sync HWDGE queue behind wfc-even/wq-even; cast on gpsimd
            # (emitted after the AG trigger so it doesn't delay it).
            wk_sb = wpool.tile([128, NJ * D], BF16, tag="wko", name="wk_sb")
            for s_ in range(NJ):
                stg = hw_w_dma(wk, "wk", s_, nc.sync)
                nc.gpsimd.tensor_copy(wk_sb[:, s_ * D:(s_ + 1) * D], stg[:])

            fc_group([(5, strips[5]), (6, strips[6])])
            fused_epilogue(5)
            fused_epilogue(6)
            fc_group([(7, strips[7])])
            fused_epilogue(7)

            for tl in range(8):
                k_tile(tl, wk_sb)

            # wv: scalar HWDGE queue + scalar-engine casts (emitted after the
            # k loop so the k-phase rms activations aren't blocked).
            wv_sb = wpool.tile([128, NJ * D], BF16, tag="wqv", name="wv_sb")
            for s_ in range(NJ):
                stg = hw_w_dma(wv, "wv", s_, nc.scalar)
                nc.scalar.copy(wv_sb[:, s_ * D:(s_ + 1) * D], stg[:])
            # wo: SWDGE cast behind the strips (WAR on wk's slot is fine --
            # wk dies at k_tile(7)).
            wo_sb = wpool.tile([128, NJ * D], BF16, tag="wko", name="wo_sb")
            nc.gpsimd.dma_start(
                wo_sb[:].rearrange("p (s c) -> p s c", s=NJ),
                wo.ap().rearrange("(s p) c -> p s c", p=128))

            for i in range(NQT):
                attn_tile(i)
            nc.gpsimd.collective_compute(
                "ReduceScatter", OP.add,
                replica_groups=GROUPS,
                ins=[rs_in.opt()],
                outs=[rs_out.opt()],
            )

            # ---------------- epilogue for own 128 queries --------------
            fo = scr_f.tile([QPC, D + 1], F16, tag="osb", bufs=2)
            nc.sync.dma_start(fo[:], rs_out[:])
            linv = stat.tile([128, 1], F32, tag="linv")
            nc.vector.reciprocal(linv[:], fo[:, D:D + 1])
            ao = scr_bf.tile([128, D], BF16, tag="tmb")
            nc.vector.tensor_scalar(ao[:], fo[:, 0:D], linv[:], None, OP.mult)
            aoT = scr_bf.tile([128, D], BF16, tag="tmb")
            transpose_full(aoT, ao[:])
            vps = mmps.tile([128, D], F32, tag="mm")
            for j in range(NJ):
                for h in range(2):
                    nc.tensor.matmul(
                        vps[:, h * 512:(h + 1) * 512],
                        aoT[:, j * 128:(j + 1) * 128],
                        wv_sb[:, j * D + h * 512: j * D + h * 512 + 512],
                        start=(j == 0),
                        stop=(j == NJ - 1),
                    )
            vb = scr_bf.tile([128, D], BF16, tag="tmb")
            nc.vector.tensor_copy(vb[:], vps[:])
            vbT = scr_bf.tile([128, D], BF16, tag="tmb")
            transpose_full(vbT, vb[:])
            zps = mmps.tile([128, D], F32, tag="mm")
            for j in range(NJ):
                for h in range(2):
                    nc.tensor.matmul(
                        zps[:, h * 512:(h + 1) * 512],
                        vbT[:, j * 128:(j + 1) * 128],
                        wo_sb[:, j * D + h * 512: j * D + h * 512 + 512],
                        start=(j == 0),
                        stop=(j == NJ - 1),
                    )
            hh = scr_f.tile([128, D], F32, tag="sqscr")
            nc.vector.tensor_add(hh[:], qs_bf[:], zps[:])
            orstd = rms_stats(hh[:])
            yv = scr_f.tile([128, D], F32, tag="sqscr")
            nc.vector.tensor_scalar(yv[:], hh[:], orstd[:], None, OP.mult)
            if apply_norm_weights:
                nc.vector.tensor_tensor(yv[:], yv[:], won_sb[:], op=OP.mult)
            nc.sync.dma_start(out[:], yv[:])

    nc.compile()
    return nc


def _pe_table():
    half = D // 2
    inv_freq = np.exp(np.arange(half, dtype=np.float32)
                      * (-math.log(10000.0) / half))
    ang = np.arange(S, dtype=np.float32)[:, None] * inv_freq
    return np.concatenate([np.sin(ang), np.cos(ang)], axis=-1).astype(np.float32)


def make_in_maps(np_inputs, apply_w=False):
    hid = np.asarray(np_inputs["hidden_states"], np.float32)
    pos = np.asarray(np_inputs["context_positions"])
    Wfc = np.ascontiguousarray(np.asarray(np_inputs["W_fc"], np.float32))
    Wq = np.ascontiguousarray(np.asarray(np_inputs["Wq"], np.float32))
    Wk = np.ascontiguousarray(np.asarray(np_inputs["Wk"], np.float32))
    Wv = np.ascontiguousarray(np.asarray(np_inputs["Wv"], np.float32))
    Wo = np.ascontiguousarray(np.asarray(np_inputs["Wo"], np.float32))

    x = hid.reshape(B, S, LD)
    p = np.clip(pos.astype(np.int64), 0, S - 1)          # [B, N]
    PE = _pe_table()

    iota_np = np.tile(np.arange(CH, dtype=np.float16), (128, 1))
    ident_np = np.eye(128, dtype=np.float32).astype(ml_dtypes.bfloat16)

    in_maps = []
    for c in range(NC):
        b, g = divmod(c, GRP)
        chunk0 = g * CH
        own_pos = p[b, g * QPC:(g + 1) * QPC]            # [128]
        # strip layout [128 part, kk, 128 tok]: elem (p,kk,t) = x^T[kk*128+p, t]
        x3 = np.empty((NT, 128, NKK, 128), np.float32)
        xq_cols = x[b, own_pos].T                        # [3072, 128]
        x3[0] = xq_cols.reshape(NKK, 128, 128).transpose(1, 0, 2)
        for t in range(8):
            r0 = chunk0 + t * 128
            x3[1 + t] = (x[b, r0:r0 + 128, :].T
                         .reshape(NKK, 128, 128).transpose(1, 0, 2))
        peq_a = np.ascontiguousarray(PE[own_pos]).astype(ml_dtypes.bfloat16)
        # pet: per k-tile tl, rows [tl*1024:(tl+1)*1024] = PE.T of its 128
        # positions -> [8, 1024 d, 128 t] flattened
        petT = PE[chunk0:chunk0 + CH].T                  # [1024 d, 1024 t]
        pet_a = np.ascontiguousarray(
            petT.reshape(D, 8, 128).transpose(1, 0, 2)
            .reshape(8 * D, 128)).astype(ml_dtypes.bfloat16)
        # thr col i = positions of batch-b query tile i, minus chunk start
        thr_a = np.ascontiguousarray(
            (p[b].astype(np.float32) - chunk0).reshape(NQT, QPC).T)
        m = {
            "x3": x3.reshape(NT * 128, NKK * 128),
            "wfc": Wfc, "wq": Wq, "wk": Wk, "wv": Wv, "wo": Wo,
            "pet": pet_a, "peq": peq_a, "thr": thr_a,
            "iota": iota_np, "ident": ident_np,
        }
        if apply_w:
            m["whn"] = np.tile(np.asarray(np_inputs["w_hidden_norm"], np.float32), (128, 1))
            m["wqn"] = np.tile(np.asarray(np_inputs["w_q_norm"], np.float32), (128, 1))
            m["wkn"] = np.tile(np.asarray(np_inputs["w_k_norm"], np.float32), (128, 1))
            m["won"] = np.tile(np.asarray(np_inputs["w_out_norm"], np.float32), (128, 1))
        in_maps.append(m)
    return in_maps


def assemble_out(results):
    y = np.zeros((B, N, D), np.float32)
    for c in range(NC):
        b, g = divmod(c, GRP)
        y[b, g * QPC:(g + 1) * QPC] = results[c]["out"]
    return y


def kernel(**inputs) -> np.ndarray:
    w_h = np.asarray(inputs["w_hidden_norm"], np.float32)
    w_q = np.asarray(inputs["w_q_norm"], np.float32)
    w_k = np.asarray(inputs["w_k_norm"], np.float32)
    w_o = np.asarray(inputs["w_out_norm"], np.float32)
    apply_w = not (np.all(w_h == 1) and np.all(w_q == 1)
                   and np.all(w_k == 1) and np.all(w_o == 1))

    key = ("nc", apply_w)
    if key not in _CACHE:
        _CACHE[key] = _build(apply_w)
    nc = _CACHE[key]

    in_maps = make_in_maps(inputs, apply_w)

    trace = os.environ.get("KERNEL_TRACE", "0") == "1"
    if trace:
        try:
            import axon_prof
            axon_prof.install()
        except Exception:
            trace = False
    res = run_bass_kernel_spmd(nc, in_maps, list(range(NC)), trace=trace)
    global LAST_EXEC_NS
    LAST_EXEC_NS = res.exec_time_ns

    return assemble_out(res.results).astype(np.float32)


LAST_EXEC_NS = None


# revision 16
# speedup vs baseline: 1.1169x; 1.1169x over previous
"""Trainium2 Bass kernel for nn_CHSHistoryCrossAttentionFusion (8 NeuronCores, SPMD).

Decomposition (hardcoded for B=2, S=4096, L=3, D=1024, N=512, 8 cores):
  - Batch-split history sharding: cores 0-3 own batch 0, cores 4-7 batch 1;
    core c owns key positions [(c%4)*1024, (c%4+1)*1024) of its batch and
    computes fused/K for that chunk from its x strips.
  - Queries: core c owns the 128 queries (c%4)*128..+128 of its batch; it
    computes their fused/Q projection, then a 4-core-group AllGather (groups
    [0..3] / [4..7] run concurrently) replicates the *pre-transposed* Q so
    every core scores all 512 of its batch's queries against its K chunk.
  - Flash-style partial softmax per chunk WITHOUT max subtraction (Q/K are
    RMS-normalized so scores are bounded); causal mask applied additively
    before exp; exp carries a constant -ln(256) prescale so the (o,l)
    partials fit fp16.  Wv is factored OUT of the partial accumulation:
    partials are attn@fused (not attn@V), combined by one fp16 4-core-group
    ReduceScatter, and Wv/Wo are applied to the core's own 128 queries only.
  - x is supplied host-side as 9 pre-transposed, partition-reblocked
    [128, 24*128] strips (8 history tiles + 1 gathered query tile) so the
    fc matmul needs no on-chip transposes; strips stream f32->bf16 through
    the single SWDGE cast queue (wo follows them).  Dense weights stream
    f32 on the two HWDGE queues (sync+scalar) and are cast to bf16 on
    vector/gpsimd/scalar, ordered so wfc+wq land first (fc + AllGather
    critical path), then wk for the K phase, then wv/wo for the epilogue.
    Cast/collective emission is placed so no in-order engine queue blocks
    on data it doesn't need yet; the Q epilogue is split so the fps_t8
    PSUM drains before group B while the Wq matmul waits for wq casts.
  - All matmuls bf16 (fp32 accumulate); f32 in/out.
Host-side work is layout/indexing only (the sinusoidal position table is
host-precomputed as in the reference construction, shipped bf16).
"""

import math
import os

import numpy as np

try:
    import ml_dtypes
except ImportError:  # pragma: no cover
    ml_dtypes = None

import concourse.bacc as bacc
import concourse.mybir as mybir
import concourse.tile as tile
import concourse.tile_utils as tile_utils
from concourse.bass_utils import run_bass_kernel_spmd

# cayman has 208 KiB/partition usable; the default constant leaves 16 KiB idle
tile_utils.max_sbuf_usage = 208 * 1024

F32 = mybir.dt.float32
F16 = mybir.dt.float16
BF16 = mybir.dt.bfloat16
AF = mybir.ActivationFunctionType
OP = mybir.AluOpType

B, S, L, D = 2, 4096, 3, 1024
N = 512
NC = 8
GRP = 4                   # collective group size (one batch per group)
CH = S // GRP             # 1024 keys per core (one batch)
LD = L * D                # 3072
QPC = 128                 # queries owned per core
NQT = N // QPC            # 4 query tiles per batch
NKK = LD // 128           # 24 contraction slices over 3072
NJ = D // 128             # 8 contraction slices over 1024
NT = 9                    # 8 history tiles + 1 query tile
RMS_EPS = 1e-6
SCALE = D ** -0.5
MASK_NEG = -60000.0      # fits f16; exp(MASK_NEG * SCALE) == 0
EXP_BIAS = -math.log(256.0)

_CACHE = {}


def _build(apply_norm_weights: bool):
    nc = bacc.Bacc("TRN2", target_bir_lowering=False, num_devices=NC)

    # ---------------- I/O ----------------
    x3 = nc.dram_tensor("x3", [NT * 128, NKK * 128], F32, kind="ExternalInput")
    wfc = nc.dram_tensor("wfc", [LD, D], F32, kind="ExternalInput")
    wq = nc.dram_tensor("wq", [D, D], F32, kind="ExternalInput")
    wk = nc.dram_tensor("wk", [D, D], F32, kind="ExternalInput")
    wv = nc.dram_tensor("wv", [D, D], F32, kind="ExternalInput")
    wo = nc.dram_tensor("wo", [D, D], F32, kind="ExternalInput")
    pet = nc.dram_tensor("pet", [8 * D, 128], BF16, kind="ExternalInput")
    peq = nc.dram_tensor("peq", [QPC, D], BF16, kind="ExternalInput")
    thr = nc.dram_tensor("thr", [128, NQT], F32, kind="ExternalInput")
    iota = nc.dram_tensor("iota", [128, CH], F16, kind="ExternalInput")
    ident = nc.dram_tensor("ident", [128, 128], BF16, kind="ExternalInput")
    if apply_norm_weights:
        whn = nc.dram_tensor("whn", [128, D], F32, kind="ExternalInput")
        wqn = nc.dram_tensor("wqn", [128, D], F32, kind="ExternalInput")
        wkn = nc.dram_tensor("wkn", [128, D], F32, kind="ExternalInput")
        won = nc.dram_tensor("won", [128, D], F32, kind="ExternalInput")
    out = nc.dram_tensor("out", [QPC, D], F32, kind="ExternalOutput")

    GROUPS = [[0, 1, 2, 3], [4, 5, 6, 7]]

    with tile.TileContext(nc) as tc:
        with (
            tc.tile_pool(name="dram", bufs=1, space="DRAM") as dram,
            tc.tile_pool(name="const", bufs=1) as constp,
            tc.tile_pool(name="stat", bufs=6) as stat,
            tc.tile_pool(name="wpool", bufs=1) as wpool,
            tc.tile_pool(name="stage", bufs=3) as stagep,
            tc.tile_pool(name="strip", bufs=4) as stripp,
            tc.tile_pool(name="base", bufs=1) as base,
            tc.tile_pool(name="scr_bf", bufs=2) as scr_bf,
            tc.tile_pool(name="scr_f", bufs=2) as scr_f,
            tc.tile_pool(name="mmps", bufs=3, space="PSUM") as mmps,
            tc.tile_pool(name="trps", bufs=2, space="PSUM") as trps,
        ):
            # collective bounce buffers
            ag_in = dram.tile([D, QPC], BF16)
            ag_out = dram.tile([GRP * D, QPC], BF16)
            rs_in = dram.tile([N, D + 1], F16)
            rs_out = dram.tile([QPC, D + 1], F16)

            # ---- small constants (sync queue, land first) ----
            id_sb = constp.tile([128, 128], BF16)
            nc.sync.dma_start(id_sb[:], ident[:])
            iota_sb = constp.tile([128, CH], F16)
            nc.sync.dma_start(iota_sb[:], iota[:])
            thr_sb = constp.tile([128, NQT], F32)
            nc.sync.dma_start(thr_sb[:], thr[:])
            eps_sb = constp.tile([128, 1], F32)
            nc.vector.memset(eps_sb[:], RMS_EPS)
            ebias_sb = constp.tile([128, 1], F32)
            nc.vector.memset(ebias_sb[:], EXP_BIAS)
            if apply_norm_weights:
                whn_sb = constp.tile([128, D], F32)
                nc.sync.dma_start(whn_sb[:], whn[:])
                wqn_sb = constp.tile([128, D], F32)
                nc.sync.dma_start(wqn_sb[:], wqn[:])
                wkn_sb = constp.tile([128, D], F32)
                nc.sync.dma_start(wkn_sb[:], wkn[:])
                won_sb = constp.tile([128, D], F32)
                nc.sync.dma_start(won_sb[:], won[:])

            # peq (bf16, tiny) first on the scalar HWDGE queue
            peq_bf = wpool.tile([QPC, D], BF16)
            nc.scalar.dma_start(peq_bf[:], peq.ap())

            # ---- bulk loads ----
            # SWDGE (gpsimd cast queue): x strips, query strip first.
            strips = [None] * 8

            def load_strip(t, name):
                st = stripp.tile([128, NKK * 128], BF16, tag="strip",
                                 name=name)
                idx = 0 if t is None else 1 + t
                nc.gpsimd.dma_start(
                    st[:], x3.ap()[idx * 128:(idx + 1) * 128, :])
                return st

            # SWDGE: 3 strips first, then the wfc tail slices (cast-DMA,
            # no staging), then the remaining strips, then wo (emitted later).
            strip_q = load_strip(None, "strip_q")
            strips[0] = load_strip(0, "strip0")
            strips[1] = load_strip(1, "strip1")

            NKK_HW = 12           # wfc slices 0..11 via HWDGE, 12..23 SWDGE
            wfc_bf = wpool.tile([128, NKK * D], BF16)
            nc.gpsimd.dma_start(
                wfc_bf[:, NKK_HW * D:].rearrange("p (s c) -> p s c",
                                                 s=NKK - NKK_HW),
                wfc.ap()[NKK_HW * 128:, :]
                .rearrange("(s p) c -> p s c", p=128))

            for t in range(2, 8):
                strips[t] = load_strip(t, f"strip{t}")

            def wfc_slice(kk):
                stg = stagep.tile([128, D], F32, tag="stg", name=f"wfst{kk}")
                eng = nc.sync if kk % 2 == 0 else nc.scalar
                eng.dma_start(stg[:], wfc.ap()[kk * 128:(kk + 1) * 128, :])
                nc.vector.tensor_copy(wfc_bf[:, kk * D:(kk + 1) * D], stg[:])

            for kk in range(NKK_HW):
                wfc_slice(kk)

            def hw_w_dma(src, nm, s_, eng):
                stg = stagep.tile([128, D], F32, tag="stg", name=f"{nm}st{s_}")
                eng.dma_start(stg[:], src.ap()[s_ * 128:(s_ + 1) * 128, :])
                return stg

            # wq: split across both HWDGE queues; cast on vector.
            wq_sb = wpool.tile([128, NJ * D], BF16, tag="wqv", name="wq_sb")
            for s_ in range(NJ):
                stg = hw_w_dma(wq, "wq", s_,
                               nc.sync if s_ % 2 == 0 else nc.scalar)
                nc.vector.tensor_copy(wq_sb[:, s_ * D:(s_ + 1) * D], stg[:])

            # persistent activations
            fusedT = base.tile([128, NJ * CH], BF16, name="fusedT")
            fusedT_v = fusedT[:].rearrange("p (j t) -> p j t", j=NJ)
            fnat = base.tile([128, 8 * D], BF16, name="fnat")
            qs_bf = base.tile([QPC, D], BF16)
            kT = base.tile([128, NJ * CH], BF16, name="kT")
            kT_v = kT[:].rearrange("p (j t) -> p j t", j=NJ)

            def rms_stats(src_ap):
                sq = scr_f.tile([128, D], F32, tag="sqscr")
                ssq = stat.tile([128, 1], F32, tag="ssq")
                nc.scalar.activation(sq[:], src_ap, AF.Square, accum_out=ssq[:])
                std = stat.tile([128, 1], F32, tag="std")
                nc.scalar.activation(std[:], ssq[:], AF.Sqrt, scale=1.0 / D,
                                     bias=eps_sb[:])
                rstd = stat.tile([128, 1], F32, tag="rstd")
                nc.vector.reciprocal(rstd[:], std[:])
                return rstd

            def transpose_to(dst_ap_3d, src_tile_ap, jlist):
                """PE-transpose 128x128 blocks into dst 3d view [128,len,128]."""
                ps = trps.tile([128, 512], BF16, tag="trp")
                for u, j in enumerate(jlist):
                    nc.tensor.transpose(
                        ps[:, u * 128:(u + 1) * 128],
                        src_tile_ap[:, j * 128:(j + 1) * 128],
                        id_sb[:],
                    )
                nc.vector.tensor_copy(
                    dst_ap_3d,
                    ps[:].rearrange("p (u x) -> p u x", u=len(jlist)),
                )

            def transpose_full(dst_tile, src_ap):
                """dst[:, j*128:+128] = src[:, j*128:+128].T for all 8 j."""
                dst_v = dst_tile[:].rearrange("p (g x) -> p g x", g=2)
                for g in range(2):
                    transpose_to(
                        dst_v[:, g:g + 1, :].rearrange("p g x -> p (g x)")
                        .rearrange("p (u x) -> p u x", u=4),
                        src_ap,
                        [g * 4 + u for u in range(4)],
                    )

            # ---------------- phase 1: fc matmul ----------
            fps_tiles = {}

            def fc_group(tiles):
                """kk-outer over a group (used while wfc streams in)."""
                for key, _ in tiles:
                    fps_tiles[key] = mmps.tile([128, D], F32, tag="mm",
                                               name=f"fps{key}")
                for kk in range(NKK):
                    for key, st in tiles:
                        fps = fps_tiles[key]
                        for h in range(2):
                            nc.tensor.matmul(
                                fps[:, h * 512:(h + 1) * 512],
                                st[:, kk * 128:(kk + 1) * 128],
                                wfc_bf[:, kk * D + h * 512: kk * D + h * 512 + 512],
                                start=(kk == 0),
                                stop=(kk == NKK - 1),
                            )

            def fused_epilogue(t):
                fps = fps_tiles[t]
                rstd = rms_stats(fps[:])
                fb = fnat[:, t * D:(t + 1) * D]
                nc.vector.tensor_scalar(fb, fps[:], rstd[:], None, OP.mult)
                if apply_norm_weights:
                    nc.vector.tensor_tensor(fb, fb, whn_sb[:], op=OP.mult)
                for g in range(2):
                    transpose_to(
                        fusedT_v[:, g * 4:(g + 1) * 4,
                                 t * 128:(t + 1) * 128],
                        fb,
                        [g * 4 + u for u in range(4)],
                    )

            qep = {}

            def q_epilogue_a():
                """Drains fps_t8: rms + positioned qT; no wq dependency."""
                fps = fps_tiles["t8"]
                rstd = rms_stats(fps[:])
                nc.vector.tensor_scalar(qs_bf[:], fps[:], rstd[:], None,
                                        OP.mult)
                if apply_norm_weights:
                    nc.vector.tensor_tensor(qs_bf[:], qs_bf[:], whn_sb[:],
                                            op=OP.mult)
                qhb = scr_bf.tile([128, D], BF16, tag="tmb")
                nc.vector.tensor_scalar(qhb[:], fps[:], rstd[:], None, OP.mult)
                if apply_norm_weights:
                    nc.vector.tensor_tensor(qhb[:], qhb[:], whn_sb[:],
                                            op=OP.mult)
                nc.vector.tensor_add(qhb[:], qhb[:], peq_bf[:])
                qht = scr_bf.tile([128, D], BF16, tag="tmb")
                transpose_full(qht, qhb[:])
                qep["qht"] = qht

            def q_epilogue_b():
                """Wq matmul + rms + own-qT AllGather (needs wq casts)."""
                qht = qep["qht"]
                qps = mmps.tile([128, D], F32, tag="mm", name="qps")
                for j in range(NJ):
                    for h in range(2):
                        nc.tensor.matmul(
                            qps[:, h * 512:(h + 1) * 512],
                            qht[:, j * 128:(j + 1) * 128],
                            wq_sb[:, j * D + h * 512: j * D + h * 512 + 512],
                            start=(j == 0),
                            stop=(j == NJ - 1),
                        )
                qrstd = rms_stats(qps[:])
                qb = scr_bf.tile([128, D], BF16, tag="tmb")
                nc.vector.tensor_scalar(qb[:], qps[:], qrstd[:], None, OP.mult)
                if apply_norm_weights:
                    nc.vector.tensor_tensor(qb[:], qb[:], wqn_sb[:],
                                            op=OP.mult)
                qt = scr_bf.tile([128, D], BF16, tag="tmb")
                transpose_full(qt, qb[:])
                nc.sync.dma_start(
                    ag_in.rearrange("(j p) q -> p j q", p=128),
                    qt[:].rearrange("p (j q) -> p j q", j=NJ))
                nc.gpsimd.collective_compute(
                    "AllGather", OP.bypass,
                    replica_groups=GROUPS,
                    ins=[ag_in.opt()],
                    outs=[ag_out.opt()],
                )

            # -------- phase 2 helpers --------
            def k_tile(tl, wk_sb):
                pts = scr_bf.tile([128, NJ * 128], BF16, tag="pts", bufs=2)
                nc.sync.dma_start(
                    pts[:].rearrange("p (j t) -> p j t", j=NJ),
                    pet.ap()[tl * D:(tl + 1) * D, :]
                    .rearrange("(j p) t -> p j t", p=128))
                khb = scr_bf.tile([128, NJ * 128], BF16, tag="khb", bufs=2)
                nc.vector.tensor_add(
                    khb[:].rearrange("p (j x) -> p j x", j=NJ),
                    fusedT_v[:, :, tl * 128:(tl + 1) * 128],
                    pts[:].rearrange("p (j t) -> p j t", j=NJ),
                )
                kps = mmps.tile([128, D], F32, tag="mm")
                for j in range(NJ):
                    for h in range(2):
                        nc.tensor.matmul(
                            kps[:, h * 512:(h + 1) * 512],
                            khb[:, j * 128:(j + 1) * 128],
                            wk_sb[:, j * D + h * 512: j * D + h * 512 + 512],
                            start=(j == 0),
                            stop=(j == NJ - 1),
                        )
                krstd = rms_stats(kps[:])
                kb = scr_bf.tile([128, D], BF16, tag="tmb")
                nc.vector.tensor_scalar(kb[:], kps[:], krstd[:], None, OP.mult)
                if apply_norm_weights:
                    nc.vector.tensor_tensor(kb[:], kb[:], wkn_sb[:],
                                            op=OP.mult)
                for g in range(2):
                    transpose_to(
                        kT_v[:, g * 4:(g + 1) * 4, tl * 128:(tl + 1) * 128],
                        kb[:],
                        [g * 4 + u for u in range(4)],
                    )

            def attn_tile(i):
                # load pre-transposed q tile i straight from the AllGather
                qTt = scr_bf.tile([128, NJ * 128], BF16, tag="qTt", bufs=2,
                                  name=f"qT{i}")
                nc.sync.dma_start(
                    qTt[:].rearrange("p (j q) -> p j q", j=NJ),
                    ag_out[i * D:(i + 1) * D, :]
                    .rearrange("(j p) q -> p j q", p=128))
                sps = mmps.tile([128, CH], F32, tag="mm")
                for j in range(NJ):
                    for h in range(2):
                        nc.tensor.matmul(
                            sps[:, h * 512:(h + 1) * 512],
                            qTt[:, j * 128:(j + 1) * 128],
                            kT[:, j * CH + h * 512: j * CH + h * 512 + 512],
                            start=(j == 0),
                            stop=(j == NJ - 1),
                        )
                mb = scr_f.tile([128, CH], F16, tag="mb", bufs=2)
                nc.vector.tensor_scalar(mb[:], iota_sb[:],
                                        thr_sb[:, i:i + 1], MASK_NEG,
                                        OP.is_gt, OP.mult)
                nc.vector.tensor_add(mb[:], mb[:], sps[:])
                o_sb = scr_f.tile([128, D + 1], F16, tag="osb", bufs=2)
                lacc = stat.tile([128, 1], F32, tag="lacc")
                probs = scr_bf.tile([128, CH], BF16, tag="probs", bufs=1)
                nc.scalar.activation(probs[:], mb[:], AF.Exp, scale=SCALE,
                                     bias=ebias_sb[:], accum_out=lacc[:])
                nc.scalar.copy(o_sb[:, D:D + 1], lacc[:])
                pT = scr_bf.tile([128, NJ * 128], BF16, tag="pT", bufs=1)
                pT_v = pT[:].rearrange("p (u x) -> p u x", u=NJ)
                for g in range(2):
                    transpose_to(
                        pT_v[:, g * 4:(g + 1) * 4, :],
                        probs[:, g * 512:(g + 1) * 512],
                        list(range(4)),
                    )
                ops_ = mmps.tile([128, D], F32, tag="mm")
                for u in range(8):
                    for h in range(2):
                        nc.tensor.matmul(
                            ops_[:, h * 512:(h + 1) * 512],
                            pT[:, u * 128:(u + 1) * 128],
                            fnat[:, u * D + h * 512: u * D + h * 512 + 512],
                            start=(u == 0),
                            stop=(u == 7),
                        )
                nc.vector.tensor_copy(o_sb[:, 0:D], ops_[:])
                nc.scalar.dma_start(rs_in[i * 128:(i + 1) * 128, :], o_sb[:])

            # ---------------- emission schedule ----------------
            # fc groups 3+3+2+1 (PSUM: 3 tiles max + trps + qps slot reuse).
            fc_group([("t8", strip_q), (0, strips[0]), (1, strips[1])])
            q_epilogue_a()        # drains fps_t8 (no wq dependency)
            fused_epilogue(0)
            fused_epilogue(1)
            fc_group([(2, strips[2]), (3, strips[3]), (4, strips[4])])
            fused_epilogue(2)
            q_epilogue_b()        # Wq matmul + AllGather trigger (gpsimd)
            fused_epilogue(3)
            fused_epilogue(4)

            # wk: sync HWDGE queue behind wfc-even/wq-even; cast on vector
            # (emitted after the AG trigger so it doesn't delay it).
            wk_sb = wpool.tile([128, NJ * D], BF16, tag="wko", name="wk_sb")
            for s_ in range(NJ):
                stg = hw_w_dma(wk, "wk", s_, nc.sync)
                nc.vector.tensor_copy(wk_sb[:, s_ * D:(s_ + 1) * D], stg[:])

            fc_group([(5, strips[5]), (6, strips[6])])
            fused_epilogue(5)
            fused_epilogue(6)
            fc_group([(7, strips[7])])
            fused_epilogue(7)

            for tl in range(8):
                k_tile(tl, wk_sb)

            # wv: scalar HWDGE queue + vector casts (emitted after the
            # k loop so the k-phase work isn't blocked).
            wv_sb = wpool.tile([128, NJ * D], BF16, tag="wqv", name="wv_sb")
            for s_ in range(NJ):
                stg = hw_w_dma(wv, "wv", s_, nc.scalar)
                nc.vector.tensor_copy(wv_sb[:, s_ * D:(s_ + 1) * D], stg[:])
            # wo: SWDGE cast behind the strips (WAR on wk's slot is fine --
            # wk dies at k_tile(7)).
            wo_sb = wpool.tile([128, NJ * D], BF16, tag="wko", name="wo_sb")
            nc.gpsimd.dma_start(
                wo_sb[:].rearrange("p (s c) -> p s c", s=NJ),
                wo.ap().rearrange("(s p) c -> p s c", p=128))

            for i in range(NQT):
                attn_tile(i)
            nc.gpsimd.collective_compute(
                "ReduceScatter", OP.add,
                replica_groups=GROUPS,
                ins=[rs_in.opt()],
                outs=[rs_out.opt()],
            )

            # ---------------- epilogue for own 128 queries --------------
            fo = scr_f.tile([QPC, D + 1], F16, tag="osb", bufs=2)
            nc.sync.dma_start(fo[:], rs_out[:])
            linv = stat.tile([128, 1], F32, tag="linv")
            nc.vector.reciprocal(linv[:], fo[:, D:D + 1])
            ao = scr_bf.tile([128, D], BF16, tag="tmb")
            nc.vector.tensor_scalar(ao[:], fo[:, 0:D], linv[:], None, OP.mult)
            aoT = scr_bf.tile([128, D], BF16, tag="tmb")
            transpose_full(aoT, ao[:])
            vps = mmps.tile([128, D], F32, tag="mm")
            for j in range(NJ):
                for h in range(2):
                    nc.tensor.matmul(
                        vps[:, h * 512:(h + 1) * 512],
                        aoT[:, j * 128:(j + 1) * 128],
                        wv_sb[:, j * D + h * 512: j * D + h * 512 + 512],
                        start=(j == 0),
                        stop=(j == NJ - 1),
                    )
            vb = scr_bf.tile([128, D], BF16, tag="tmb")
            nc.vector.tensor_copy(vb[:], vps[:])
            vbT = scr_bf.tile([128, D], BF16, tag="tmb")
            transpose_full(vbT, vb[:])
            zps = mmps.tile([128, D], F32, tag="mm")
            for j in range(NJ):
                for h in range(2):
                    nc.tensor.matmul(
                        zps[:, h * 512:(h + 1) * 512],
                        vbT[:, j * 128:(j + 1) * 128],
                        wo_sb[:, j * D + h * 512: j * D + h * 512 + 512],
                        start=(j == 0),
                        stop=(j == NJ - 1),
                    )
            hh = scr_f.tile([128, D], F32, tag="sqscr")
            nc.vector.tensor_add(hh[:], qs_bf[:], zps[:])
            orstd = rms_stats(hh[:])
            yv = scr_f.tile([128, D], F32, tag="sqscr")
            nc.vector.tensor_scalar(yv[:], hh[:], orstd[:], None, OP.mult)
            if apply_norm_weights:
                nc.vector.tensor_tensor(yv[:], yv[:], won_sb[:], op=OP.mult)
            nc.sync.dma_start(out[:], yv[:])

    nc.compile()
    return nc


def _pe_table():
    half = D // 2
    inv_freq = np.exp(np.arange(half, dtype=np.float32)
                      * (-math.log(10000.0) / half))
    ang = np.arange(S, dtype=np.float32)[:, None] * inv_freq
    return np.concatenate([np.sin(ang), np.cos(ang)], axis=-1).astype(np.float32)


def make_in_maps(np_inputs, apply_w=False):
    hid = np.asarray(np_inputs["hidden_states"], np.float32)
    pos = np.asarray(np_inputs["context_positions"])
    Wfc = np.ascontiguousarray(np.asarray(np_inputs["W_fc"], np.float32))
    Wq = np.ascontiguousarray(np.asarray(np_inputs["Wq"], np.float32))
    Wk = np.ascontiguousarray(np.asarray(np_inputs["Wk"], np.float32))
    Wv = np.ascontiguousarray(np.asarray(np_inputs["Wv"], np.float32))
    Wo = np.ascontiguousarray(np.asarray(np_inputs["Wo"], np.float32))

    x = hid.reshape(B, S, LD)
    p = np.clip(pos.astype(np.int64), 0, S - 1)          # [B, N]
    PE = _pe_table()

    iota_np = np.tile(np.arange(CH, dtype=np.float16), (128, 1))
    ident_np = np.eye(128, dtype=np.float32).astype(ml_dtypes.bfloat16)

    in_maps = []
    for c in range(NC):
        b, g = divmod(c, GRP)
        chunk0 = g * CH
        own_pos = p[b, g * QPC:(g + 1) * QPC]            # [128]
        # strip layout [128 part, kk, 128 tok]: elem (p,kk,t) = x^T[kk*128+p, t]
        x3 = np.empty((NT, 128, NKK, 128), np.float32)
        xq_cols = x[b, own_pos].T                        # [3072, 128]
        x3[0] = xq_cols.reshape(NKK, 128, 128).transpose(1, 0, 2)
        for t in range(8):
            r0 = chunk0 + t * 128
            x3[1 + t] = (x[b, r0:r0 + 128, :].T
                         .reshape(NKK, 128, 128).transpose(1, 0, 2))
        peq_a = np.ascontiguousarray(PE[own_pos]).astype(ml_dtypes.bfloat16)
        # pet: per k-tile tl, rows [tl*1024:(tl+1)*1024] = PE.T of its 128
        # positions -> [8, 1024 d, 128 t] flattened
        petT = PE[chunk0:chunk0 + CH].T                  # [1024 d, 1024 t]
        pet_a = np.ascontiguousarray(
            petT.reshape(D, 8, 128).transpose(1, 0, 2)
            .reshape(8 * D, 128)).astype(ml_dtypes.bfloat16)
        # thr col i = positions of batch-b query tile i, minus chunk start
        thr_a = np.ascontiguousarray(
            (p[b].astype(np.float32) - chunk0).reshape(NQT, QPC).T)
        m = {
            "x3": x3.reshape(NT * 128, NKK * 128),
            "wfc": Wfc, "wq": Wq, "wk": Wk, "wv": Wv, "wo": Wo,
            "pet": pet_a, "peq": peq_a, "thr": thr_a,
            "iota": iota_np, "ident": ident_np,
        }
        if apply_w:
            m["whn"] = np.tile(np.asarray(np_inputs["w_hidden_norm"], np.float32), (128, 1))
            m["wqn"] = np.tile(np.asarray(np_inputs["w_q_norm"], np.float32), (128, 1))
            m["wkn"] = np.tile(np.asarray(np_inputs["w_k_norm"], np.float32), (128, 1))
            m["won"] = np.tile(np.asarray(np_inputs["w_out_norm"], np.float32), (128, 1))
        in_maps.append(m)
    return in_maps


def assemble_out(results):
    y = np.zeros((B, N, D), np.float32)
    for c in range(NC):
        b, g = divmod(c, GRP)
        y[b, g * QPC:(g + 1) * QPC] = results[c]["out"]
    return y


def kernel(**inputs) -> np.ndarray:
    w_h = np.asarray(inputs["w_hidden_norm"], np.float32)
    w_q = np.asarray(inputs["w_q_norm"], np.float32)
    w_k = np.asarray(inputs["w_k_norm"], np.float32)
    w_o = np.asarray(inputs["w_out_norm"], np.float32)
    apply_w = not (np.all(w_h == 1) and np.all(w_q == 1)
                   and np.all(w_k == 1) and np.all(w_o == 1))

    key = ("nc", apply_w)
    if key not in _CACHE:
        _CACHE[key] = _build(apply_w)
    nc = _CACHE[key]

    in_maps = make_in_maps(inputs, apply_w)

    trace = os.environ.get("KERNEL_TRACE", "0") == "1"
    if trace:
        try:
            import axon_prof
            axon_prof.install()
        except Exception:
            trace = False
    res = run_bass_kernel_spmd(nc, in_maps, list(range(NC)), trace=trace)
    global LAST_EXEC_NS
    LAST_EXEC_NS = res.exec_time_ns

    return assemble_out(res.results).astype(np.float32)


LAST_EXEC_NS = None


# revision 25
# speedup vs baseline: 1.1796x; 1.0561x over previous
"""Trainium2 Bass kernel for nn_CHSHistoryCrossAttentionFusion (8 NeuronCores, SPMD).

Decomposition (hardcoded for B=2, S=4096, L=3, D=1024, N=512, 8 cores):
  - Batch-split history sharding: cores 0-3 own batch 0, cores 4-7 batch 1;
    core c owns key positions [(c%4)*1024, (c%4+1)*1024) of its batch and
    computes fused/K for that chunk from its x strips.
  - Queries: core c owns the 128 queries (c%4)*128..+128 of its batch; it
    computes their fused/Q projection, then a 4-core-group AllGather (groups
    [0..3] / [4..7] run concurrently) replicates the *pre-transposed* Q so
    every core scores all 512 of its batch's queries against its K chunk.
  - Flash-style partial softmax per chunk WITHOUT max subtraction (Q/K are
    RMS-normalized so scores are bounded); causal mask applied additively
    before exp; exp carries a constant -ln(256) prescale so the (o,l)
    partials fit fp16.  Wv is factored OUT of the partial accumulation:
    partials are attn@fused (not attn@V), combined by one fp16 4-core-group
    ReduceScatter, and Wv/Wo are applied to the core's own 128 queries only.
  - x is supplied host-side as 9 pre-transposed, partition-reblocked
    [128, 24*128] strips (8 history tiles + 1 gathered query tile) so the
    fc matmul needs no on-chip transposes; strips stream f32->bf16 through
    the single SWDGE cast queue (wo follows them).  Dense weights stream
    f32 on the two HWDGE queues (sync+scalar) and are cast to bf16 on
    vector/gpsimd/scalar, ordered so wfc+wq land first (fc + AllGather
    critical path), then wk for the K phase, then wv/wo for the epilogue.
    Cast/collective emission is placed so no in-order engine queue blocks
    on data it doesn't need yet; the Q epilogue is split so the fps_t8
    PSUM drains before group B while the Wq matmul waits for wq casts.
  - All matmuls bf16 (fp32 accumulate); f32 in/out.
Host-side work is layout/indexing only (the sinusoidal position table is
host-precomputed as in the reference construction, shipped bf16).
"""

import math
import os

import numpy as np

try:
    import ml_dtypes
except ImportError:  # pragma: no cover
    ml_dtypes = None

import concourse.bacc as bacc
import concourse.mybir as mybir
import concourse.tile as tile
import concourse.tile_utils as tile_utils
from concourse.bass_utils import run_bass_kernel_spmd

# cayman has 208 KiB/partition usable; the default constant leaves 16 KiB idle
tile_utils.max_sbuf_usage = 208 * 1024

F32 = mybir.dt.float32
F16 = mybir.dt.float16
BF16 = mybir.dt.bfloat16
AF = mybir.ActivationFunctionType
OP = mybir.AluOpType

B, S, L, D = 2, 4096, 3, 1024
N = 512
NC = 8
GRP = 4                   # collective group size (one batch per group)
CH = S // GRP             # 1024 keys per core (one batch)
LD = L * D                # 3072
QPC = 128                 # queries owned per core
NQT = N // QPC            # 4 query tiles per batch
NKK = LD // 128           # 24 contraction slices over 3072
NJ = D // 128             # 8 contraction slices over 1024
NT = 9                    # 8 history tiles + 1 query tile
RMS_EPS = 1e-6
SCALE = D ** -0.5
MASK_NEG = -60000.0      # fits f16; exp(MASK_NEG * SCALE) == 0
EXP_BIAS = -math.log(256.0)

_CACHE = {}


def _build(apply_norm_weights: bool):
    nc = bacc.Bacc("TRN2", target_bir_lowering=False, num_devices=NC)

    # ---------------- I/O ----------------
    x3 = nc.dram_tensor("x3", [NT * 128, NKK * 128], F32, kind="ExternalInput")
    wfc = nc.dram_tensor("wfc", [LD, D], F32, kind="ExternalInput")
    wq = nc.dram_tensor("wq", [D, D], F32, kind="ExternalInput")
    wk = nc.dram_tensor("wk", [D, D], F32, kind="ExternalInput")
    wvt = nc.dram_tensor("wvt", [D, D], F32, kind="ExternalInput")
    wo = nc.dram_tensor("wo", [D, D], F32, kind="ExternalInput")
    pet = nc.dram_tensor("pet", [8 * D, 128], BF16, kind="ExternalInput")
    peq = nc.dram_tensor("peq", [QPC, D], BF16, kind="ExternalInput")
    thr = nc.dram_tensor("thr", [128, NQT], F32, kind="ExternalInput")
    iota = nc.dram_tensor("iota", [128, CH], F16, kind="ExternalInput")
    ident = nc.dram_tensor("ident", [128, 128], BF16, kind="ExternalInput")
    if apply_norm_weights:
        whn = nc.dram_tensor("whn", [128, D], F32, kind="ExternalInput")
        wqn = nc.dram_tensor("wqn", [128, D], F32, kind="ExternalInput")
        wkn = nc.dram_tensor("wkn", [128, D], F32, kind="ExternalInput")
        won = nc.dram_tensor("won", [128, D], F32, kind="ExternalInput")
    out = nc.dram_tensor("out", [QPC, D], F32, kind="ExternalOutput")

    GROUPS = [[0, 1, 2, 3], [4, 5, 6, 7]]

    with tile.TileContext(nc) as tc:
        with (
            tc.tile_pool(name="dram", bufs=1, space="DRAM") as dram,
            tc.tile_pool(name="const", bufs=1) as constp,
            tc.tile_pool(name="stat", bufs=6) as stat,
            tc.tile_pool(name="wpool", bufs=1) as wpool,
            tc.tile_pool(name="stage", bufs=4) as stagep,
            tc.tile_pool(name="strip", bufs=3) as stripp,
            tc.tile_pool(name="base", bufs=1) as base,
            tc.tile_pool(name="scr_bf", bufs=2) as scr_bf,
            tc.tile_pool(name="scr_f", bufs=2) as scr_f,
            tc.tile_pool(name="mmps", bufs=3, space="PSUM") as mmps,
            tc.tile_pool(name="trps", bufs=2, space="PSUM") as trps,
        ):
            # collective bounce buffers
            ag_in = dram.tile([D, QPC], BF16)
            ag_out = dram.tile([GRP * D, QPC], BF16)
            rs_in = dram.tile([N, D + 1], F16)
            rs_out = dram.tile([QPC, D + 1], F16)

            # ---- small constants (sync queue, land first) ----
            id_sb = constp.tile([128, 128], BF16)
            nc.sync.dma_start(id_sb[:], ident[:])
            iota_sb = constp.tile([128, CH], F16)
            nc.sync.dma_start(iota_sb[:], iota[:])
            thr_sb = constp.tile([128, NQT], F32)
            nc.sync.dma_start(thr_sb[:], thr[:])
            eps_sb = constp.tile([128, 1], F32)
            nc.vector.memset(eps_sb[:], RMS_EPS)
            ebias_sb = constp.tile([128, 1], F32)
            nc.vector.memset(ebias_sb[:], EXP_BIAS)
            if apply_norm_weights:
                whn_sb = constp.tile([128, D], F32)
                nc.sync.dma_start(whn_sb[:], whn[:])
                wqn_sb = constp.tile([128, D], F32)
                nc.sync.dma_start(wqn_sb[:], wqn[:])
                wkn_sb = constp.tile([128, D], F32)
                nc.sync.dma_start(wkn_sb[:], wkn[:])
                won_sb = constp.tile([128, D], F32)
                nc.sync.dma_start(won_sb[:], won[:])

            # peq (bf16, tiny) first on the scalar HWDGE queue
            peq_bf = wpool.tile([QPC, D], BF16)
            nc.scalar.dma_start(peq_bf[:], peq.ap())

            # ---- bulk loads ----
            # SWDGE (gpsimd cast queue): x strips, query strip first.
            strips = [None] * 8

            def load_strip(t, name):
                st = stripp.tile([128, NKK * 128], BF16, tag="strip",
                                 name=name)
                idx = 0 if t is None else 1 + t
                nc.gpsimd.dma_start(
                    st[:], x3.ap()[idx * 128:(idx + 1) * 128, :])
                return st

            # SWDGE: 3 strips first, then the wfc tail slices (cast-DMA,
            # no staging), then the remaining strips, then wo (emitted later).
            strip_q = load_strip(None, "strip_q")
            strips[0] = load_strip(0, "strip0")
            strips[1] = load_strip(1, "strip1")

            NKK_HW = 12           # wfc slices 0..11 via HWDGE, 12..23 SWDGE
            wfc_bf = wpool.tile([128, NKK * D], BF16)
            for c0 in range(NKK_HW, NKK, 3):   # 4 chunks of 3 slices
                nc.gpsimd.dma_start(
                    wfc_bf[:, c0 * D:(c0 + 3) * D]
                    .rearrange("p (s c) -> p s c", s=3),
                    wfc.ap()[c0 * 128:(c0 + 3) * 128, :]
                    .rearrange("(s p) c -> p s c", p=128))

            for t in range(2, 8):
                strips[t] = load_strip(t, f"strip{t}")

            def wfc_slice(kk):
                stg = stagep.tile([128, D], F32, tag="stg", name=f"wfst{kk}")
                eng = nc.sync if kk % 2 == 0 else nc.scalar
                eng.dma_start(stg[:], wfc.ap()[kk * 128:(kk + 1) * 128, :])
                nc.vector.tensor_copy(wfc_bf[:, kk * D:(kk + 1) * D], stg[:])

            for kk in range(NKK_HW):
                wfc_slice(kk)

            def hw_w_dma(src, nm, s_, eng):
                stg = stagep.tile([128, D], F32, tag="stg", name=f"{nm}st{s_}")
                eng.dma_start(stg[:], src.ap()[s_ * 128:(s_ + 1) * 128, :])
                return stg

            # wq: split across both HWDGE queues; cast on vector.
            wq_sb = wpool.tile([128, NJ * D], BF16, tag="wqv", name="wq_sb")
            for s_ in range(NJ):
                stg = hw_w_dma(wq, "wq", s_,
                               nc.sync if s_ % 2 == 0 else nc.scalar)
                nc.vector.tensor_copy(wq_sb[:, s_ * D:(s_ + 1) * D], stg[:])

            # persistent activations (fusedT's slot is recycled for the
            # folded Wv@Wo product after the k phase)
            fusedT = base.tile([128, NJ * CH], BF16, tag="ftw", bufs=1,
                               name="fusedT")
            fusedT_v = fusedT[:].rearrange("p (j t) -> p j t", j=NJ)
            fnat = base.tile([128, 8 * D], BF16, name="fnat")
            qs_bf = base.tile([QPC, D], BF16)
            kT = base.tile([128, NJ * CH], BF16, name="kT")
            kT_v = kT[:].rearrange("p (j t) -> p j t", j=NJ)

            def rms_stats(src_ap):
                sq = scr_f.tile([128, D], F32, tag="sqscr")
                ssq = stat.tile([128, 1], F32, tag="ssq")
                nc.scalar.activation(sq[:], src_ap, AF.Square, accum_out=ssq[:])
                std = stat.tile([128, 1], F32, tag="std")
                nc.scalar.activation(std[:], ssq[:], AF.Sqrt, scale=1.0 / D,
                                     bias=eps_sb[:])
                rstd = stat.tile([128, 1], F32, tag="rstd")
                nc.vector.reciprocal(rstd[:], std[:])
                return rstd

            def transpose_to(dst_ap_3d, src_tile_ap, jlist):
                """PE-transpose 128x128 blocks into dst 3d view [128,len,128]."""
                ps = trps.tile([128, 512], BF16, tag="trp")
                for u, j in enumerate(jlist):
                    nc.tensor.transpose(
                        ps[:, u * 128:(u + 1) * 128],
                        src_tile_ap[:, j * 128:(j + 1) * 128],
                        id_sb[:],
                    )
                nc.vector.tensor_copy(
                    dst_ap_3d,
                    ps[:].rearrange("p (u x) -> p u x", u=len(jlist)),
                )

            def transpose_full(dst_tile, src_ap):
                """dst[:, j*128:+128] = src[:, j*128:+128].T for all 8 j."""
                dst_v = dst_tile[:].rearrange("p (g x) -> p g x", g=2)
                for g in range(2):
                    transpose_to(
                        dst_v[:, g:g + 1, :].rearrange("p g x -> p (g x)")
                        .rearrange("p (u x) -> p u x", u=4),
                        src_ap,
                        [g * 4 + u for u in range(4)],
                    )

            # ---------------- phase 1: fc matmul ----------
            fps_tiles = {}

            def fc_group(tiles):
                """kk-outer over a group (used while wfc streams in)."""
                for key, _ in tiles:
                    fps_tiles[key] = mmps.tile([128, D], F32, tag="mm",
                                               name=f"fps{key}")
                for kk in range(NKK):
                    for key, st in tiles:
                        fps = fps_tiles[key]
                        for h in range(2):
                            nc.tensor.matmul(
                                fps[:, h * 512:(h + 1) * 512],
                                st[:, kk * 128:(kk + 1) * 128],
                                wfc_bf[:, kk * D + h * 512: kk * D + h * 512 + 512],
                                start=(kk == 0),
                                stop=(kk == NKK - 1),
                            )

            def fused_epilogue(t):
                fps = fps_tiles[t]
                rstd = rms_stats(fps[:])
                fb = fnat[:, t * D:(t + 1) * D]
                nc.vector.tensor_scalar(fb, fps[:], rstd[:], None, OP.mult)
                if apply_norm_weights:
                    nc.vector.tensor_tensor(fb, fb, whn_sb[:], op=OP.mult)
                for g in range(2):
                    transpose_to(
                        fusedT_v[:, g * 4:(g + 1) * 4,
                                 t * 128:(t + 1) * 128],
                        fb,
                        [g * 4 + u for u in range(4)],
                    )

            qep = {}

            def q_epilogue_a():
                """Drains fps_t8: rms + positioned qT; no wq dependency."""
                fps = fps_tiles["t8"]
                rstd = rms_stats(fps[:])
                nc.vector.tensor_scalar(qs_bf[:], fps[:], rstd[:], None,
                                        OP.mult)
                if apply_norm_weights:
                    nc.vector.tensor_tensor(qs_bf[:], qs_bf[:], whn_sb[:],
                                            op=OP.mult)
                qhb = scr_bf.tile([128, D], BF16, tag="tmb")
                nc.vector.tensor_scalar(qhb[:], fps[:], rstd[:], None, OP.mult)
                if apply_norm_weights:
                    nc.vector.tensor_tensor(qhb[:], qhb[:], whn_sb[:],
                                            op=OP.mult)
                nc.vector.tensor_add(qhb[:], qhb[:], peq_bf[:])
                qht = scr_bf.tile([128, D], BF16, tag="tmb")
                transpose_full(qht, qhb[:])
                qep["qht"] = qht

            def q_epilogue_b():
                """Wq matmul + rms + own-qT AllGather (needs wq casts)."""
                qht = qep["qht"]
                qps = mmps.tile([128, D], F32, tag="mm", name="qps")
                for j in range(NJ):
                    for h in range(2):
                        nc.tensor.matmul(
                            qps[:, h * 512:(h + 1) * 512],
                            qht[:, j * 128:(j + 1) * 128],
                            wq_sb[:, j * D + h * 512: j * D + h * 512 + 512],
                            start=(j == 0),
                            stop=(j == NJ - 1),
                        )
                qrstd = rms_stats(qps[:])
                qb = scr_bf.tile([128, D], BF16, tag="tmb")
                nc.vector.tensor_scalar(qb[:], qps[:], qrstd[:], None, OP.mult)
                if apply_norm_weights:
                    nc.vector.tensor_tensor(qb[:], qb[:], wqn_sb[:],
                                            op=OP.mult)
                qt = scr_bf.tile([128, D], BF16, tag="tmb")
                transpose_full(qt, qb[:])
                nc.sync.dma_start(
                    ag_in.rearrange("(j p) q -> p j q", p=128),
                    qt[:].rearrange("p (j q) -> p j q", j=NJ))
                nc.gpsimd.collective_compute(
                    "AllGather", OP.bypass,
                    replica_groups=GROUPS,
                    ins=[ag_in.opt()],
                    outs=[ag_out.opt()],
                )

            # -------- phase 2 helpers (software-pipelined mm/post halves) ----
            kps_t = {}

            def k_mm(tl, wk_sb):
                pts = scr_bf.tile([128, NJ * 128], BF16, tag="pts", bufs=2)
                nc.sync.dma_start(
                    pts[:].rearrange("p (j t) -> p j t", j=NJ),
                    pet.ap()[tl * D:(tl + 1) * D, :]
                    .rearrange("(j p) t -> p j t", p=128))
                khb = scr_bf.tile([128, NJ * 128], BF16, tag="khb", bufs=2)
                nc.vector.tensor_add(
                    khb[:].rearrange("p (j x) -> p j x", j=NJ),
                    fusedT_v[:, :, tl * 128:(tl + 1) * 128],
                    pts[:].rearrange("p (j t) -> p j t", j=NJ),
                )
                kps = mmps.tile([128, D], F32, tag="mm")
                kps_t[tl] = kps
                for j in range(NJ):
                    for h in range(2):
                        nc.tensor.matmul(
                            kps[:, h * 512:(h + 1) * 512],
                            khb[:, j * 128:(j + 1) * 128],
                            wk_sb[:, j * D + h * 512: j * D + h * 512 + 512],
                            start=(j == 0),
                            stop=(j == NJ - 1),
                        )

            def k_post(tl):
                kps = kps_t.pop(tl)
                krstd = rms_stats(kps[:])
                kb = scr_bf.tile([128, D], BF16, tag="tmb")
                nc.vector.tensor_scalar(kb[:], kps[:], krstd[:], None, OP.mult)
                if apply_norm_weights:
                    nc.vector.tensor_tensor(kb[:], kb[:], wkn_sb[:],
                                            op=OP.mult)
                for g in range(2):
                    transpose_to(
                        kT_v[:, g * 4:(g + 1) * 4, tl * 128:(tl + 1) * 128],
                        kb[:],
                        [g * 4 + u for u in range(4)],
                    )

            sps_t = {}

            def attn_mm(i):
                # load pre-transposed q tile i straight from the AllGather
                qTt = scr_bf.tile([128, NJ * 128], BF16, tag="qTt", bufs=2,
                                  name=f"qT{i}")
                nc.sync.dma_start(
                    qTt[:].rearrange("p (j q) -> p j q", j=NJ),
                    ag_out[i * D:(i + 1) * D, :]
                    .rearrange("(j p) q -> p j q", p=128))
                sps = mmps.tile([128, CH], F32, tag="mm")
                sps_t[i] = sps
                for j in range(NJ):
                    for h in range(2):
                        nc.tensor.matmul(
                            sps[:, h * 512:(h + 1) * 512],
                            qTt[:, j * 128:(j + 1) * 128],
                            kT[:, j * CH + h * 512: j * CH + h * 512 + 512],
                            start=(j == 0),
                            stop=(j == NJ - 1),
                        )

            def attn_post(i):
                sps = sps_t.pop(i)
                mb = scr_f.tile([128, CH], F16, tag="mb", bufs=2)
                nc.vector.tensor_scalar(mb[:], iota_sb[:],
                                        thr_sb[:, i:i + 1], MASK_NEG,
                                        OP.is_gt, OP.mult)
                nc.vector.tensor_add(mb[:], mb[:], sps[:])
                o_sb = scr_f.tile([128, D + 1], F16, tag="osb", bufs=2)
                lacc = stat.tile([128, 1], F32, tag="lacc")
                probs = scr_bf.tile([128, CH], BF16, tag="probs", bufs=1)
                nc.scalar.activation(probs[:], mb[:], AF.Exp, scale=SCALE,
                                     bias=ebias_sb[:], accum_out=lacc[:])
                nc.scalar.copy(o_sb[:, D:D + 1], lacc[:])
                pT = scr_bf.tile([128, NJ * 128], BF16, tag="pT", bufs=2)
                pT_v = pT[:].rearrange("p (u x) -> p u x", u=NJ)
                for g in range(2):
                    transpose_to(
                        pT_v[:, g * 4:(g + 1) * 4, :],
                        probs[:, g * 512:(g + 1) * 512],
                        list(range(4)),
                    )
                ops_ = mmps.tile([128, D], F32, tag="mm")
                for u in range(8):
                    for h in range(2):
                        nc.tensor.matmul(
                            ops_[:, h * 512:(h + 1) * 512],
                            pT[:, u * 128:(u + 1) * 128],
                            fnat[:, u * D + h * 512: u * D + h * 512 + 512],
                            start=(u == 0),
                            stop=(u == 7),
                        )
                nc.vector.tensor_copy(o_sb[:, 0:D], ops_[:])
                nc.scalar.dma_start(rs_in[i * 128:(i + 1) * 128, :], o_sb[:])

            # ---------------- emission schedule ----------------
            # fc groups 3+3+2+1 (PSUM: 3 tiles max + trps + qps slot reuse).
            fc_group([("t8", strip_q), (0, strips[0]), (1, strips[1])])
            q_epilogue_a()        # drains fps_t8 (no wq dependency)
            fused_epilogue(0)
            fused_epilogue(1)
            fc_group([(2, strips[2]), (3, strips[3]), (4, strips[4])])
            fused_epilogue(2)
            q_epilogue_b()        # Wq matmul + AllGather trigger (gpsimd)
            fused_epilogue(3)
            fused_epilogue(4)

            # wk: sync HWDGE queue behind wfc-even/wq-even; cast on vector
            # (emitted after the AG trigger so it doesn't delay it).
            wk_sb = wpool.tile([128, NJ * D], BF16, tag="wko", name="wk_sb")
            for s_ in range(NJ):
                stg = hw_w_dma(wk, "wk", s_, nc.sync)
                nc.vector.tensor_copy(wk_sb[:, s_ * D:(s_ + 1) * D], stg[:])

            fc_group([(5, strips[5]), (6, strips[6])])
            fused_epilogue(5)
            fused_epilogue(6)
            fc_group([(7, strips[7])])
            fused_epilogue(7)

            # k phase, software-pipelined: mm(t+1) overlaps post(t)
            k_mm(0, wk_sb)
            k_mm(1, wk_sb)
            for tl in range(8):
                k_post(tl)
                if tl + 2 < 8:
                    k_mm(tl + 2, wk_sb)

            # wvt (Wv^T): scalar HWDGE queue + vector casts (emitted after
            # the k loop so the k-phase work isn't blocked).
            wvt_sb = wpool.tile([128, NJ * D], BF16, tag="wqv", name="wvt_sb")
            for s_ in range(NJ):
                stg = hw_w_dma(wvt, "wvt", s_, nc.scalar)
                nc.vector.tensor_copy(wvt_sb[:, s_ * D:(s_ + 1) * D], stg[:])
            # wo: SWDGE cast behind the strips (WAR on wk's slot is fine --
            # wk dies at k_post(7)).
            wo_sb = wpool.tile([128, NJ * D], BF16, tag="wko", name="wo_sb")
            nc.gpsimd.dma_start(
                wo_sb[:].rearrange("p (s c) -> p s c", s=NJ),
                wo.ap().rearrange("(s p) c -> p s c", p=128))

            # attention, software-pipelined
            attn_mm(0)
            attn_mm(1)
            for i in range(NQT):
                attn_post(i)
                if i + 2 < NQT:
                    attn_mm(i + 2)
            nc.gpsimd.collective_compute(
                "ReduceScatter", OP.add,
                replica_groups=GROUPS,
                ins=[rs_in.opt()],
                outs=[rs_out.opt()],
            )

            # W_vo = Wv @ Wo folded on-device during the ReduceScatter
            # window (PE is otherwise idle); lands in fusedT's slot.
            wvo = base.tile([128, NJ * D], BF16, tag="ftw", bufs=1,
                            name="wvo")
            for ic in range(NJ):
                wps = mmps.tile([128, D], F32, tag="mm")
                for j in range(NJ):
                    for h in range(2):
                        nc.tensor.matmul(
                            wps[:, h * 512:(h + 1) * 512],
                            wvt_sb[:, j * D + ic * 128: j * D + ic * 128 + 128],
                            wo_sb[:, j * D + h * 512: j * D + h * 512 + 512],
                            start=(j == 0),
                            stop=(j == NJ - 1),
                        )
                nc.vector.tensor_copy(wvo[:, ic * D:(ic + 1) * D], wps[:])

            # ---------------- epilogue for own 128 queries --------------
            fo = scr_f.tile([QPC, D + 1], F16, tag="osb", bufs=2)
            nc.sync.dma_start(fo[:], rs_out[:])
            linv = stat.tile([128, 1], F32, tag="linv")
            nc.vector.reciprocal(linv[:], fo[:, D:D + 1])
            ao = scr_bf.tile([128, D], BF16, tag="tmb")
            nc.vector.tensor_scalar(ao[:], fo[:, 0:D], linv[:], None, OP.mult)
            aoT = scr_bf.tile([128, D], BF16, tag="tmb")
            transpose_full(aoT, ao[:])
            zps = mmps.tile([128, D], F32, tag="mm")
            for j in range(NJ):
                for h in range(2):
                    nc.tensor.matmul(
                        zps[:, h * 512:(h + 1) * 512],
                        aoT[:, j * 128:(j + 1) * 128],
                        wvo[:, j * D + h * 512: j * D + h * 512 + 512],
                        start=(j == 0),
                        stop=(j == NJ - 1),
                    )
            hh = scr_f.tile([128, D], F32, tag="sqscr")
            nc.vector.tensor_add(hh[:], qs_bf[:], zps[:])
            orstd = rms_stats(hh[:])
            yv = scr_f.tile([128, D], F32, tag="sqscr")
            nc.vector.tensor_scalar(yv[:], hh[:], orstd[:], None, OP.mult)
            if apply_norm_weights:
                nc.vector.tensor_tensor(yv[:], yv[:], won_sb[:], op=OP.mult)
            nc.sync.dma_start(out[:], yv[:])

    nc.compile()
    return nc


def _pe_table():
    half = D // 2
    inv_freq = np.exp(np.arange(half, dtype=np.float32)
                      * (-math.log(10000.0) / half))
    ang = np.arange(S, dtype=np.float32)[:, None] * inv_freq
    return np.concatenate([np.sin(ang), np.cos(ang)], axis=-1).astype(np.float32)


def make_in_maps(np_inputs, apply_w=False):
    hid = np.asarray(np_inputs["hidden_states"], np.float32)
    pos = np.asarray(np_inputs["context_positions"])
    Wfc = np.ascontiguousarray(np.asarray(np_inputs["W_fc"], np.float32))
    Wq = np.ascontiguousarray(np.asarray(np_inputs["Wq"], np.float32))
    Wk = np.ascontiguousarray(np.asarray(np_inputs["Wk"], np.float32))
    Wvt = np.ascontiguousarray(np.asarray(np_inputs["Wv"], np.float32).T)
    Wo = np.ascontiguousarray(np.asarray(np_inputs["Wo"], np.float32))

    x = hid.reshape(B, S, LD)
    p = np.clip(pos.astype(np.int64), 0, S - 1)          # [B, N]
    PE = _pe_table()

    iota_np = np.tile(np.arange(CH, dtype=np.float16), (128, 1))
    ident_np = np.eye(128, dtype=np.float32).astype(ml_dtypes.bfloat16)

    in_maps = []
    for c in range(NC):
        b, g = divmod(c, GRP)
        chunk0 = g * CH
        own_pos = p[b, g * QPC:(g + 1) * QPC]            # [128]
        # strip layout [128 part, kk, 128 tok]: elem (p,kk,t) = x^T[kk*128+p, t]
        x3 = np.empty((NT, 128, NKK, 128), np.float32)
        xq_cols = x[b, own_pos].T                        # [3072, 128]
        x3[0] = xq_cols.reshape(NKK, 128, 128).transpose(1, 0, 2)
        for t in range(8):
            r0 = chunk0 + t * 128
            x3[1 + t] = (x[b, r0:r0 + 128, :].T
                         .reshape(NKK, 128, 128).transpose(1, 0, 2))
        peq_a = np.ascontiguousarray(PE[own_pos]).astype(ml_dtypes.bfloat16)
        # pet: per k-tile tl, rows [tl*1024:(tl+1)*1024] = PE.T of its 128
        # positions -> [8, 1024 d, 128 t] flattened
        petT = PE[chunk0:chunk0 + CH].T                  # [1024 d, 1024 t]
        pet_a = np.ascontiguousarray(
            petT.reshape(D, 8, 128).transpose(1, 0, 2)
            .reshape(8 * D, 128)).astype(ml_dtypes.bfloat16)
        # thr col i = positions of batch-b query tile i, minus chunk start
        thr_a = np.ascontiguousarray(
            (p[b].astype(np.float32) - chunk0).reshape(NQT, QPC).T)
        m = {
            "x3": x3.reshape(NT * 128, NKK * 128),
            "wfc": Wfc, "wq": Wq, "wk": Wk, "wvt": Wvt, "wo": Wo,
            "pet": pet_a, "peq": peq_a, "thr": thr_a,
            "iota": iota_np, "ident": ident_np,
        }
        if apply_w:
            m["whn"] = np.tile(np.asarray(np_inputs["w_hidden_norm"], np.float32), (128, 1))
            m["wqn"] = np.tile(np.asarray(np_inputs["w_q_norm"], np.float32), (128, 1))
            m["wkn"] = np.tile(np.asarray(np_inputs["w_k_norm"], np.float32), (128, 1))
            m["won"] = np.tile(np.asarray(np_inputs["w_out_norm"], np.float32), (128, 1))
        in_maps.append(m)
    return in_maps


def assemble_out(results):
    y = np.zeros((B, N, D), np.float32)
    for c in range(NC):
        b, g = divmod(c, GRP)
        y[b, g * QPC:(g + 1) * QPC] = results[c]["out"]
    return y


def kernel(**inputs) -> np.ndarray:
    w_h = np.asarray(inputs["w_hidden_norm"], np.float32)
    w_q = np.asarray(inputs["w_q_norm"], np.float32)
    w_k = np.asarray(inputs["w_k_norm"], np.float32)
    w_o = np.asarray(inputs["w_out_norm"], np.float32)
    apply_w = not (np.all(w_h == 1) and np.all(w_q == 1)
                   and np.all(w_k == 1) and np.all(w_o == 1))

    key = ("nc", apply_w)
    if key not in _CACHE:
        _CACHE[key] = _build(apply_w)
    nc = _CACHE[key]

    in_maps = make_in_maps(inputs, apply_w)

    trace = os.environ.get("KERNEL_TRACE", "0") == "1"
    if trace:
        try:
            import axon_prof
            axon_prof.install()
        except Exception:
            trace = False
    res = run_bass_kernel_spmd(nc, in_maps, list(range(NC)), trace=trace)
    global LAST_EXEC_NS
    LAST_EXEC_NS = res.exec_time_ns

    return assemble_out(res.results).astype(np.float32)


LAST_EXEC_NS = None
